# revision 27
# baseline (speedup 1.0000x reference)
"""DLRM dot-interaction kernel for Trainium2 (8 NeuronCores, batch-sharded).

Per sample b: T = concat(dense[b], embs[b]) -> [27, 128]; Z = T @ T^T;
output = strict upper triangle of Z -> [351] fp32 (computed fp16 on-chip,
upcast on host; fp16 quantization of outputs adds ~2.5e-4 rel err).

Per-core plan (2048 samples, 16 blocks of 128):
  - SWDGE cast-DMA loads (fp32 -> fp16), b-major; blocks 0-2 in two half
    tiles each so the PE can start during the pipeline-fill phase, later
    blocks one tile each, prefetched ~3 ahead (measured: finer chunking or
    HWDGE fp32 for block 0 only delays first-data arrival -- the SDMA
    round-robins across queued transfers).
  - PE transposes each [128 b, 128 d] feature slab (transpose-mode fp16,
    ~107ns, LDW-paced at the fixed 1.2 GHz NX column rate).  Four tp
    groups per block; each group's PSUM->SBUF copy is split DVE-front/
    ACT-back so the tp slot drains fast.
  - Per-sample gram matmuls: lhsT = [128 d, 26] (features 0..25), rhs =
    [128 d, 26] (features 1..26) strided slices of f-major Tt -- the
    strict upper triangle only needs rows m<26 and cols n>0.  out -> PSUM
    zp[32*g + m, q*32 + (n-1)] fp32 (sample s = q*4 + g), ~34ns/sample
    (serial LDW+MM; ldw-opt, FWL, and any LDW-column sharing are
    unavailable/ineffective in this toolchain -- measured).  Gram halves
    are emitted interleaved between the next block's transpose groups so
    the PE instruction stream consumption stays below the 16KB
    instruction-page prefetch bandwidth (dense gram bursts outrun it).
  - One DVE StreamTranspose per block swaps m<->q inside each quadrant:
    PSUM -> SBUF Zb fp32.  Triu pack per block: 26 contiguous-run copies
    cast fp32 -> fp16 into Pk (ACT takes the 6 longest runs, DVE the
    rest -- DVE small-op issue rate is ~80ns, ACT ~310ns); ONE
    128-partition HWDGE DMA with 702B runs writes out[b, :] fp16
    (partition (g,q) -> row q*4+g).  Per-block packing keeps the
    post-last-matmul drain chain short (ST + pack + one DMA).
"""

import numpy as np

B, NUM_EMBS, D = 16384, 26, 128
N_CORES = 8
BC = B // N_CORES  # 2048 samples per core
BLK = 128          # samples per block
NF = NUM_EMBS + 1  # 27 features
FP = 32            # feature pitch in the Z PSUM tile
NPAIR = NF * (NF - 1) // 2  # 351
GROUPS = (7, 7, 7, 6)    # transpose groups per block

_CACHE = {}


def build(bc=BC):
    import concourse.bacc as bacc
    import concourse.mybir as mybir
    from concourse.tile import TileContext
    from concourse.masks import make_identity

    fp16 = mybir.dt.float16
    fp32 = mybir.dt.float32

    nc = bacc.Bacc("TRN2", target_bir_lowering=False, debug=False)
    dense_t = nc.dram_tensor("dense", (bc, D), fp32, kind="ExternalInput")
    embs_t = nc.dram_tensor("embs", (bc, NUM_EMBS, D), fp32, kind="ExternalInput")
    out_t = nc.dram_tensor("out", (bc, NPAIR), fp16, kind="ExternalOutput")

    nblk = bc // BLK

    with TileContext(nc) as tc:
        with (
            tc.tile_pool(name="consts", bufs=1) as consts,
            tc.tile_pool(name="xin", bufs=1) as xpool,
            tc.tile_pool(name="tt", bufs=5) as ttpool,
            tc.tile_pool(name="zb", bufs=3) as zbpool,
            tc.tile_pool(name="pk", bufs=3) as pkpool,
            tc.tile_pool(name="tp", bufs=4, space="PSUM") as tppool,
            tc.tile_pool(name="zp", bufs=2, space="PSUM") as zppool,
        ):
            ident = consts.tile([128, 128], fp16)
            make_identity(nc, ident)

            dview = dense_t.ap()  # [bc, 128]
            eview = embs_t.ap().rearrange("b f d -> b (f d)")  # [bc, 3328]
            oview = out_t.ap()  # [bc, 351]

            xmap = {}    # blk -> list of (tile, f0, nf) segments
            tts = {}
            gstate = {}  # blk -> (zp, Ttr, zb, Pk)

            def emit_load(blk):
                b0 = blk * BLK
                if blk <= 2:
                    # two half-tiles: transposes start after ~half the load
                    Xa = xpool.tile([BLK, 14 * D], fp16, tag="Xh0", bufs=3, name="Xa")
                    nc.gpsimd.dma_start(out=Xa[:, 0:D], in_=dview[b0 : b0 + BLK])
                    nc.gpsimd.dma_start(
                        out=Xa[:, D:], in_=eview[b0 : b0 + BLK, : 13 * D]
                    )
                    Xb = xpool.tile([BLK, 13 * D], fp16, tag="Xh1", bufs=3, name="Xb")
                    nc.gpsimd.dma_start(out=Xb[:, :], in_=eview[b0 : b0 + BLK, 13 * D :])
                    xmap[blk] = [(Xa, 0, 14), (Xb, 14, 13)]
                else:
                    X = xpool.tile([BLK, NF * D], fp16, tag="X", bufs=6, name="X")
                    nc.gpsimd.dma_start(out=X[:, 0:D], in_=dview[b0 : b0 + BLK])
                    nc.gpsimd.dma_start(out=X[:, D:], in_=eview[b0 : b0 + BLK])
                    xmap[blk] = [(X, 0, NF)]

            def _slab(blk, f):
                for tile, f0, nf in xmap[blk]:
                    if f0 <= f < f0 + nf:
                        c0 = (f - f0) * D
                        return tile[:, c0 : c0 + D]
                raise AssertionError

            def emit_transpose_group(blk, ci):
                if ci == 0:
                    tts[blk] = ttpool.tile([128, NF * D], fp16, tag="Tt", name="Tt")
                Tt = tts[blk]
                c0 = sum(GROUPS[:ci])
                cf = GROUPS[ci]
                tp = tppool.tile([128, 7 * BLK], fp16, tag="tp", name="tp")
                for j in range(cf):
                    nc.tensor.transpose(
                        tp[:, j * BLK : (j + 1) * BLK], _slab(blk, c0 + j), ident
                    )
                # split the PSUM->SBUF copy across DVE and ACT so the tp
                # slot frees quickly even when one queue is busy
                h = (cf * BLK) // 2
                nc.vector.tensor_copy(
                    out=Tt[:, c0 * BLK : c0 * BLK + h], in_=tp[:, :h]
                )
                nc.scalar.copy(
                    Tt[:, c0 * BLK + h : (c0 + cf) * BLK], tp[:, h : cf * BLK]
                )
                if ci == len(GROUPS) - 1:
                    del xmap[blk]

            def emit_gram_half(blk, hf):
                if hf == 0:
                    Tt = tts.pop(blk)
                    Ttr = Tt.rearrange("d (f b) -> d b f", b=BLK)
                    zp = zppool.tile([128, FP * FP], fp32, tag="zp", name="zp")
                    zb = zbpool.tile([128, FP * FP], fp32, tag="Zb", name="zb")
                    Pk = pkpool.tile([128, NPAIR], fp16, tag="Pk", name="Pk")
                    gstate[blk] = (zp, Ttr, zb, Pk)
                zp, Ttr, zb, Pk = gstate[blk]
                for q in range(16 * hf, 16 * hf + 16):
                    for g in range(4):
                        s = q * 4 + g
                        nc.tensor.matmul(
                            zp[32 * g : 32 * g + NF - 1, q * FP : q * FP + NF - 1],
                            Ttr[:, s, 0 : NF - 1],
                            Ttr[:, s, 1:NF],
                            start=True,
                            stop=True,
                            tile_position=(0, 32 * g),
                        )
                if hf == 1:
                    del gstate[blk]
                    # ST: PSUM (g,m),(q,n) -> SBUF (g,q),(m,n)
                    inv = zp.rearrange("p (q n) -> p n q", n=FP)[:, 0 : NF - 1, :]
                    outv = zb.rearrange("p (m n) -> p n m", n=FP)[:, 0 : NF - 1, :]
                    nc.vector.transpose(out=outv, in_=inv)
                    # triu pack fp32 -> fp16
                    zbv = zb.rearrange("p (m n) -> p m n", n=FP)
                    off = 0
                    for m in range(NF - 1):
                        ln = NF - 1 - m
                        src = zbv[:, m, m : m + ln]
                        dst = Pk[:, off : off + ln]
                        if m < 6:
                            nc.scalar.copy(dst, src)
                        else:
                            nc.vector.tensor_copy(out=dst, in_=src)
                        off += ln
                    # one 128-partition DMA: rows b = q*4 + g
                    ov = oview[blk * BLK : (blk + 1) * BLK].rearrange(
                        "(q g) c -> g q c", g=4
                    )
                    srcp = Pk.rearrange("(g q) c -> g q c", g=4)
                    nc.sync.dma_start(out=ov, in_=srcp)

            def emit_gram_block(blk):
                emit_gram_half(blk, 0)
                emit_gram_half(blk, 1)

            # Pipeline: loads prefetch ~3 blocks ahead; gram halves of the
            # previous block interleave between transpose groups to keep
            # the PE instruction consumption flat.
            emit_load(0)
            emit_load(1)
            emit_load(2)
            emit_load(3)
            emit_load(4)
            for ci in range(4):
                emit_transpose_group(0, ci)
            for ci in range(4):
                emit_transpose_group(1, ci)
            emit_gram_block(0)
            emit_gram_block(1)
            for blk in range(2, nblk):
                if blk + 3 < nblk:
                    emit_load(blk + 3)
                for ci in range(4):
                    emit_transpose_group(blk, ci)
                    if blk >= 3 and ci % 2 == 1:
                        emit_gram_half(blk - 1, ci // 2)
            emit_gram_block(nblk - 1)

    nc.compile()
    return nc


def _get(bc=BC):
    if bc not in _CACHE:
        _CACHE[bc] = build(bc)
    return _CACHE[bc]


def kernel(dense: np.ndarray, embs: np.ndarray) -> np.ndarray:
    from concourse import bass_utils

    dense = np.ascontiguousarray(np.asarray(dense, dtype=np.float32))
    embs = np.ascontiguousarray(np.asarray(embs, dtype=np.float32))
    assert dense.shape == (B, D) and embs.shape == (B, NUM_EMBS, D)

    nc = _get()
    dsh = dense.reshape(N_CORES, BC, D)
    esh = embs.reshape(N_CORES, BC, NUM_EMBS, D)
    in_maps = [{"dense": dsh[i], "embs": esh[i]} for i in range(N_CORES)]
    res = bass_utils.run_bass_kernel_spmd(nc, in_maps, core_ids=list(range(N_CORES)))
    return np.concatenate([r["out"] for r in res.results], axis=0).astype(np.float32)


# revision 32
# speedup vs baseline: 1.0118x; 1.0118x over previous
"""DLRM dot-interaction kernel for Trainium2 (8 NeuronCores, batch-sharded).

Per sample b: T = concat(dense[b], embs[b]) -> [27, 128]; Z = T @ T^T;
output = strict upper triangle of Z -> [351] fp32 (computed fp16 on-chip,
upcast on host; fp16 quantization of outputs adds ~2.5e-4 rel err).

Per-core plan (2048 samples, 16 blocks of 128):
  - SWDGE cast-DMA loads (fp32 -> fp16), b-major; blocks 0-2 in two half
    tiles each so the PE can start during the pipeline-fill phase, later
    blocks one tile each, prefetched ~3 ahead (measured: finer chunking or
    HWDGE fp32 for block 0 only delays first-data arrival -- the SDMA
    round-robins across queued transfers).
  - PE transposes each [128 b, 128 d] feature slab (transpose-mode fp16,
    ~107ns, LDW-paced at the fixed 1.2 GHz NX column rate).  Four tp
    groups per block; each group's PSUM->SBUF copy is split DVE-front/
    ACT-back so the tp slot drains fast.
  - Per-sample gram matmuls: lhsT = [128 d, 26] (features 0..25), rhs =
    [128 d, 26] (features 1..26) strided slices of f-major Tt -- the
    strict upper triangle only needs rows m<26 and cols n>0.  out -> PSUM
    zp[32*g + m, q*32 + (n-1)] fp32 (sample s = q*4 + g), ~34ns/sample
    (serial LDW+MM; ldw-opt, FWL, and any LDW-column sharing are
    unavailable/ineffective in this toolchain -- measured).  Gram halves
    are emitted interleaved between the next block's transpose groups so
    the PE instruction stream consumption stays below the 16KB
    instruction-page prefetch bandwidth (dense gram bursts outrun it).
  - One DVE StreamTranspose per block swaps m<->q inside each quadrant:
    PSUM -> SBUF Zb fp32 (slot t of its pack group).  Triu pack: 26
    contiguous-run copies per group cast fp32 -> fp16 into Pk; HWDGE
    DMAs with 702B runs write out[b, :] fp16 (partition (g,q) -> row
    q*4+g).  Blocks 0-11 pack in 4-block groups (DVE/ACT alternating,
    per-g DMAs); the (12,13,14) group packs on ACT only and block 15 on
    DVE only, each with whole-block 128-partition DMAs, giving two
    parallel drain chains and a short post-last-matmul tail.
"""

import numpy as np

B, NUM_EMBS, D = 16384, 26, 128
N_CORES = 8
BC = B // N_CORES  # 2048 samples per core
BLK = 128          # samples per block
NF = NUM_EMBS + 1  # 27 features
FP = 32            # feature pitch in the Z PSUM tile
NPAIR = NF * (NF - 1) // 2  # 351
GROUPS = (7, 7, 7, 6)    # transpose groups per block
NPK = 4                  # max blocks per pack group

_CACHE = {}


def build(bc=BC):
    import concourse.bacc as bacc
    import concourse.mybir as mybir
    from concourse.tile import TileContext
    from concourse.masks import make_identity

    fp16 = mybir.dt.float16
    fp32 = mybir.dt.float32

    nc = bacc.Bacc("TRN2", target_bir_lowering=False, debug=False)
    dense_t = nc.dram_tensor("dense", (bc, D), fp32, kind="ExternalInput")
    embs_t = nc.dram_tensor("embs", (bc, NUM_EMBS, D), fp32, kind="ExternalInput")
    out_t = nc.dram_tensor("out", (bc, NPAIR), fp16, kind="ExternalOutput")

    nblk = bc // BLK
    # pack groups: NPK-block groups packed m-alternating on DVE/ACT with
    # per-g DMAs; the (12,13,14) group packs on ACT only (per-block DMAs)
    # and block 15 packs on DVE only (one DMA) -- two parallel drain
    # chains so the post-last-matmul tail is short.
    pgroups = []
    b = 0
    while b < nblk - NPK:
        hi = min(b + NPK, nblk - NPK)
        pgroups.append((tuple(range(b, hi)), "mid"))
        b = hi
    pgroups.append((tuple(range(nblk - NPK, nblk - 1)), "act"))
    pgroups.append(((nblk - 1,), "dve"))
    pg_of = {}
    for grp, mode in pgroups:
        for blk in grp:
            pg_of[blk] = (grp, mode)

    with TileContext(nc) as tc:
        with (
            tc.tile_pool(name="consts", bufs=1) as consts,
            tc.tile_pool(name="xin", bufs=1) as xpool,
            tc.tile_pool(name="tt", bufs=5) as ttpool,
            tc.tile_pool(name="zb", bufs=3) as zbpool,
            tc.tile_pool(name="pk", bufs=3) as pkpool,
            tc.tile_pool(name="tp", bufs=4, space="PSUM") as tppool,
            tc.tile_pool(name="zp", bufs=2, space="PSUM") as zppool,
        ):
            ident = consts.tile([128, 128], fp16)
            make_identity(nc, ident)

            dview = dense_t.ap()  # [bc, 128]
            eview = embs_t.ap().rearrange("b f d -> b (f d)")  # [bc, 3328]
            oview = out_t.ap()  # [bc, 351]

            xmap = {}    # blk -> list of (tile, f0, nf) segments
            tts = {}

            def emit_load(blk):
                b0 = blk * BLK
                if blk <= 2:
                    # two half-tiles: transposes start after ~half the load
                    Xa = xpool.tile([BLK, 14 * D], fp16, tag="Xh0", bufs=3, name="Xa")
                    nc.gpsimd.dma_start(out=Xa[:, 0:D], in_=dview[b0 : b0 + BLK])
                    nc.gpsimd.dma_start(
                        out=Xa[:, D:], in_=eview[b0 : b0 + BLK, : 13 * D]
                    )
                    Xb = xpool.tile([BLK, 13 * D], fp16, tag="Xh1", bufs=3, name="Xb")
                    nc.gpsimd.dma_start(out=Xb[:, :], in_=eview[b0 : b0 + BLK, 13 * D :])
                    xmap[blk] = [(Xa, 0, 14), (Xb, 14, 13)]
                else:
                    X = xpool.tile([BLK, NF * D], fp16, tag="X", bufs=6, name="X")
                    nc.gpsimd.dma_start(out=X[:, 0:D], in_=dview[b0 : b0 + BLK])
                    nc.gpsimd.dma_start(out=X[:, D:], in_=eview[b0 : b0 + BLK])
                    xmap[blk] = [(X, 0, NF)]

            def _slab(blk, f):
                for tile, f0, nf in xmap[blk]:
                    if f0 <= f < f0 + nf:
                        c0 = (f - f0) * D
                        return tile[:, c0 : c0 + D]
                raise AssertionError

            def emit_transpose_group(blk, ci):
                if ci == 0:
                    tts[blk] = ttpool.tile([128, NF * D], fp16, tag="Tt", name="Tt")
                Tt = tts[blk]
                c0 = sum(GROUPS[:ci])
                cf = GROUPS[ci]
                tp = tppool.tile([128, 7 * BLK], fp16, tag="tp", name="tp")
                for j in range(cf):
                    nc.tensor.transpose(
                        tp[:, j * BLK : (j + 1) * BLK], _slab(blk, c0 + j), ident
                    )
                # split the PSUM->SBUF copy across DVE and ACT so the tp
                # slot frees quickly even when one queue is busy
                h = (cf * BLK) // 2
                nc.vector.tensor_copy(
                    out=Tt[:, c0 * BLK : c0 * BLK + h], in_=tp[:, :h]
                )
                nc.scalar.copy(
                    Tt[:, c0 * BLK + h : (c0 + cf) * BLK], tp[:, h : cf * BLK]
                )
                if ci == len(GROUPS) - 1:
                    del xmap[blk]

            zbs = {}  # grp -> (zb, Pk)

            def emit_pack(zb, Pk, grp, mode):
                """Triu pack fp32 -> fp16 + out DMA for a pack group."""
                nt = len(grp)
                zbv = zb.rearrange("p (t m n) -> p t m n", t=NPK, n=FP)
                pkv = Pk.rearrange("p (t c) -> p t c", t=NPK)
                off = 0
                for m in range(NF - 1):
                    ln = NF - 1 - m
                    src = zbv[:, 0:nt, m, m : m + ln]
                    dst = pkv[:, 0:nt, off : off + ln]
                    on_act = (m % 2 == 1) if mode == "mid" else (mode == "act")
                    if on_act:
                        nc.scalar.copy(dst, src)
                    else:
                        nc.vector.tensor_copy(out=dst, in_=src)
                    off += ln
                if mode == "mid":
                    b0 = grp[0] * BLK
                    ovq = oview[b0 : b0 + nt * 128].rearrange(
                        "(t q g) c -> g q t c", t=nt, g=4
                    )
                    pkg = Pk.rearrange("(g q) (t c) -> g q t c", g=4, t=NPK)[
                        :, :, 0:nt, :
                    ]
                    for g in range(4):
                        nc.sync.dma_start(out=ovq[g], in_=pkg[g])
                else:
                    # tail groups: one 128-partition DMA per block
                    for t, blk in enumerate(grp):
                        ov = oview[blk * BLK : (blk + 1) * BLK].rearrange(
                            "(q g) c -> g q c", g=4
                        )
                        srcp = Pk[:, t * NPAIR : (t + 1) * NPAIR].rearrange(
                            "(g q) c -> g q c", g=4
                        )
                        nc.sync.dma_start(out=ov, in_=srcp)

            def emit_gram_block(blk):
                grp, mode = pg_of[blk]
                t = grp.index(blk)
                if t == 0:
                    zb = zbpool.tile([128, NPK * FP * FP], fp32, tag="Zb", name="zb")
                    Pk = pkpool.tile([128, NPK * NPAIR], fp16, tag="Pk", name="Pk")
                    zbs[grp] = (zb, Pk)
                zb, Pk = zbs[grp]
                Tt = tts.pop(blk)
                Ttr = Tt.rearrange("d (f b) -> d b f", b=BLK)
                zp = zppool.tile([128, FP * FP], fp32, tag="zp", name="zp")
                for q in range(32):
                    for g in range(4):
                        s = q * 4 + g
                        nc.tensor.matmul(
                            zp[32 * g : 32 * g + NF - 1, q * FP : q * FP + NF - 1],
                            Ttr[:, s, 0 : NF - 1],
                            Ttr[:, s, 1:NF],
                            start=True,
                            stop=True,
                            tile_position=(0, 32 * g),
                        )
                # ST: PSUM (g,m),(q,n) -> SBUF slot t, (g,q),(m,n)
                inv = zp.rearrange("p (q n) -> p n q", n=FP)[:, 0 : NF - 1, :]
                outv = zb.rearrange("p (t m n) -> p t n m", t=NPK, n=FP)[
                    :, t, 0 : NF - 1, :
                ]
                nc.vector.transpose(out=outv, in_=inv)
                if blk == grp[-1]:
                    del zbs[grp]
                    emit_pack(zb, Pk, grp, mode)

            # Pipeline: loads prefetch ~3 blocks ahead of the transposes;
            # gram matmuls lag the transposes by one block.
            emit_load(0)
            emit_load(1)
            emit_load(2)
            emit_load(3)
            emit_load(4)
            for ci in range(4):
                emit_transpose_group(0, ci)
            for ci in range(4):
                emit_transpose_group(1, ci)
            emit_gram_block(0)
            emit_gram_block(1)
            for blk in range(2, nblk):
                if blk + 3 < nblk:
                    emit_load(blk + 3)
                for ci in range(4):
                    emit_transpose_group(blk, ci)
                if blk >= 3:
                    emit_gram_block(blk - 1)
            emit_gram_block(nblk - 1)

    nc.compile()
    return nc


def _get(bc=BC):
    if bc not in _CACHE:
        _CACHE[bc] = build(bc)
    return _CACHE[bc]


def kernel(dense: np.ndarray, embs: np.ndarray) -> np.ndarray:
    from concourse import bass_utils

    dense = np.ascontiguousarray(np.asarray(dense, dtype=np.float32))
    embs = np.ascontiguousarray(np.asarray(embs, dtype=np.float32))
    assert dense.shape == (B, D) and embs.shape == (B, NUM_EMBS, D)

    nc = _get()
    dsh = dense.reshape(N_CORES, BC, D)
    esh = embs.reshape(N_CORES, BC, NUM_EMBS, D)
    in_maps = [{"dense": dsh[i], "embs": esh[i]} for i in range(N_CORES)]
    res = bass_utils.run_bass_kernel_spmd(nc, in_maps, core_ids=list(range(N_CORES)))
    return np.concatenate([r["out"] for r in res.results], axis=0).astype(np.float32)


# revision 34
# speedup vs baseline: 1.0529x; 1.0406x over previous
"""DLRM dot-interaction kernel for Trainium2 (8 NeuronCores, batch-sharded).

Per sample b: T = concat(dense[b], embs[b]) -> [27, 128]; Z = T @ T^T;
output = strict upper triangle of Z -> [351] fp32.

Per-core plan (2048 samples, 16 blocks of 128):
  - SWDGE cast-DMA loads one block at a time (fp32 -> fp16), block 0 in
    two half-tiles so the PE can start transposing ~3us in.  Per-block
    loads keep input arrival smooth: the kernel is paced by the PE with
    the input stream just barely keeping up, so 4-block load lumps would
    directly stall the in-order PE queue.
  - PE transposes each [128 b, 128 d] feature slab (transpose-mode fp16,
    LDWEIGHTS+MM pair ~107ns at the HAM-throttled 1.2 GHz clock; the HAM
    never warms for transpose-mode work, so this is the steady rate).
  - Per-sample gram matmuls: lhsT = rhs = [128 d, 27 f] strided slice of
    f-major Tt; out -> PSUM zp[32*g + m, q*32 + n] fp32, col-group
    tiling (sample s = q*4 + g), ~34ns/sample (serial LDW+MM; the
    toolchain compiles with --enable-ldw-opt=false and bass emits
    standalone InstLdweights, so LDW/MM overlap is not available).
  - One DVE StreamTranspose per block swaps m<->q inside each quadrant:
    PSUM [(g,m), (n,q)-view] -> SBUF Zb[(g,q), m*32+n] fp32, replacing a
    DRAM scratch bounce + 55k-descriptor gather with 16 instructions.
  - Triu pack: 26 contiguous-run DVE/ACT copies per pack group into
    Pk [(g,q), t*351]; HWDGE DMAs with 1404B runs write out[b, :]
    (partition (g,q) -> row q*4+g).  The last two blocks pack singly to
    shorten the drain tail.
"""

import numpy as np

B, NUM_EMBS, D = 16384, 26, 128
N_CORES = 8
BC = B // N_CORES  # 2048 samples per core
BLK = 128          # samples per block
NF = NUM_EMBS + 1  # 27 features
FP = 32            # feature pitch in the Z PSUM tile
NPAIR = NF * (NF - 1) // 2  # 351

_CACHE = {}


def build(bc=BC):
    import concourse.bacc as bacc
    import concourse.mybir as mybir
    from concourse.tile import TileContext
    from concourse.masks import make_identity

    fp16 = mybir.dt.float16
    fp32 = mybir.dt.float32

    nc = bacc.Bacc("TRN2", target_bir_lowering=False, debug=False)
    dense_t = nc.dram_tensor("dense", (bc, D), fp32, kind="ExternalInput")
    embs_t = nc.dram_tensor("embs", (bc, NUM_EMBS, D), fp32, kind="ExternalInput")
    out_t = nc.dram_tensor("out", (bc, NPAIR), fp32, kind="ExternalOutput")

    nblk = bc // BLK
    # pack groups: pairs, with the final two blocks packed singly so the
    # post-PE drain (StreamTranspose + pack + out DMA) tail is short
    pgroups = [(b, b + 1) for b in range(0, nblk - 2, 2)] + [(nblk - 2,), (nblk - 1,)]
    pg_of = {}
    for gi, grp in enumerate(pgroups):
        for b in grp:
            pg_of[b] = (gi, grp)

    with TileContext(nc) as tc:
        with (
            tc.tile_pool(name="consts", bufs=1) as consts,
            tc.tile_pool(name="xin", bufs=7) as xpool,
            tc.tile_pool(name="tt", bufs=5) as ttpool,
            tc.tile_pool(name="zb", bufs=3) as zbpool,
            tc.tile_pool(name="pk", bufs=3) as pkpool,
            tc.tile_pool(name="tp", bufs=4, space="PSUM") as tppool,
            tc.tile_pool(name="zp", bufs=2, space="PSUM") as zppool,
        ):
            ident = consts.tile([128, 128], fp16)
            make_identity(nc, ident)

            dview = dense_t.ap()  # [bc, 128]
            eview = embs_t.ap().rearrange("b f d -> b (f d)")  # [bc, 3328]
            oview = out_t.ap()  # [bc, 351]

            xmap = {}   # blk -> list of (tile, f0, nf) segments
            tts = {}
            zps = {}
            zb_t = None
            pk_t = None

            def emit_load(blk):
                b0 = blk * BLK
                if blk == 0:
                    # two half-tiles so transposes start after ~half the load
                    X0 = xpool.tile([BLK, 14 * D], fp16, tag="Xa")
                    nc.gpsimd.dma_start(out=X0[:, 0:D], in_=dview[0:BLK])
                    nc.gpsimd.dma_start(
                        out=X0[:, D:], in_=eview[0:BLK, : 13 * D]
                    )
                    X1 = xpool.tile([BLK, 13 * D], fp16, tag="Xb")
                    nc.gpsimd.dma_start(out=X1[:, :], in_=eview[0:BLK, 13 * D :])
                    xmap[blk] = [(X0, 0, 14), (X1, 14, 13)]
                else:
                    X = xpool.tile([BLK, NF * D], fp16, tag="X")
                    nc.gpsimd.dma_start(
                        out=X[:, 0:D], in_=dview[b0 : b0 + BLK]
                    )
                    nc.gpsimd.dma_start(
                        out=X[:, D:], in_=eview[b0 : b0 + BLK]
                    )
                    xmap[blk] = [(X, 0, NF)]

            def _slab(blk, f):
                for tile, f0, nf in xmap[blk]:
                    if f0 <= f < f0 + nf:
                        c0 = (f - f0) * D
                        return tile[:, c0 : c0 + D]
                raise AssertionError

            def emit_transpose_half(blk, half):
                """Half of the 27 b->d feature-slab transposes for blk."""
                if half == 0:
                    Tt = ttpool.tile([128, NF * D], fp16, tag="Tt")
                    tts[blk] = Tt
                Tt = tts[blk]
                for ci in (0, 1) if half == 0 else (2, 3):
                    c0 = ci * 7
                    cf = min(7, NF - c0)
                    tp = tppool.tile([128, 7 * BLK], fp16, tag="tp")
                    for j in range(cf):
                        nc.tensor.transpose(
                            tp[:, j * BLK : (j + 1) * BLK],
                            _slab(blk, c0 + j),
                            ident,
                        )
                    dst = Tt[:, c0 * BLK : (c0 + cf) * BLK]
                    src = tp[:, : cf * BLK]
                    if ci % 2 == 0:
                        nc.vector.tensor_copy(out=dst, in_=src)
                    else:
                        nc.scalar.copy(dst, src)
                if half == 1:
                    del xmap[blk]

            def emit_gram_half(blk, half):
                """Half (64 samples) of the per-sample gram matmuls; the
                second half is followed by the DVE StreamTranspose."""
                Tt = tts[blk]
                Ttr = Tt.rearrange("d (f b) -> d b f", b=BLK)
                if half == 0:
                    zp = zppool.tile([128, FP * FP], fp32, tag="zp")
                    zps[blk] = zp
                zp = zps[blk]
                for q in range(16 * half, 16 * half + 16):
                    for g in range(4):
                        s = q * 4 + g
                        nc.tensor.matmul(
                            zp[32 * g : 32 * g + NF, q * FP : q * FP + NF],
                            Ttr[:, s, :],
                            Ttr[:, s, :],
                            start=True,
                            stop=True,
                            tile_position=(0, 32 * g),
                        )
                if half == 1:
                    del tts[blk]
                    gi, grp = pg_of[blk]
                    t = grp.index(blk)
                    zpt = zps.pop(blk)
                    inv = zpt.rearrange("p (q n) -> p n q", n=FP)[:, 0:NF, :]
                    outv = zb_t.rearrange(
                        "p (t m n) -> p t n m", t=len(grp), n=FP
                    )[:, t, 0:NF, :]
                    nc.vector.transpose(out=outv, in_=inv)

            def gram_pre(blk):
                nonlocal zb_t
                gi, grp = pg_of[blk]
                if blk == grp[0]:
                    zb_t = zbpool.tile(
                        [128, len(grp) * FP * FP], fp32, tag="Zb"
                    )

            def gram_post(blk):
                gi, grp = pg_of[blk]
                if blk != grp[-1]:
                    return
                npk = len(grp)
                zbp = zb_t.rearrange("p (t m n) -> p t m n", t=npk, n=FP)
                Pk = pkpool.tile([128, npk * NPAIR], fp32, tag="Pk")
                pkp = Pk.rearrange("p (t c) -> p t c", t=npk)
                off = 0
                for m in range(NF - 1):
                    ln = NF - 1 - m
                    src = zbp[:, :, m, m + 1 : NF]
                    dst = pkp[:, :, off : off + ln]
                    if m % 2 == 0:
                        nc.vector.tensor_copy(out=dst, in_=src)
                    else:
                        nc.scalar.copy(dst, src)
                    off += ln
                b0 = grp[0] * BLK
                ovq = oview[b0 : b0 + npk * BLK].rearrange(
                    "(t q g) c -> g q t c", t=npk, g=4
                )
                pk4 = pkp.rearrange("(g q) t c -> g q t c", g=4)
                for g in range(4):
                    nc.sync.dma_start(out=ovq[g], in_=pk4[g])

            # Pipeline: loads prefetch ~3 blocks ahead of the transposes;
            # gram matmuls lag the transposes by one block so the PE queue
            # always has a gram burst banked behind a possibly-input-
            # stalled transpose burst.
            def emit_gram_block(blk):
                gram_pre(blk)
                emit_gram_half(blk, 0)
                emit_gram_half(blk, 1)
                gram_post(blk)

            for blk in range(5):
                emit_load(blk)
            emit_transpose_half(0, 0)
            emit_transpose_half(0, 1)
            emit_transpose_half(1, 0)
            emit_transpose_half(1, 1)
            emit_gram_block(0)
            emit_gram_block(1)
            for blk in range(2, nblk):
                if blk + 3 < nblk:
                    emit_load(blk + 3)
                emit_transpose_half(blk, 0)
                emit_transpose_half(blk, 1)
                if blk >= 3:
                    emit_gram_block(blk - 1)
            emit_gram_block(nblk - 1)

    nc.compile()
    return nc


def _get(bc=BC):
    if bc not in _CACHE:
        _CACHE[bc] = build(bc)
    return _CACHE[bc]


def kernel(dense: np.ndarray, embs: np.ndarray) -> np.ndarray:
    from concourse import bass_utils

    dense = np.ascontiguousarray(np.asarray(dense, dtype=np.float32))
    embs = np.ascontiguousarray(np.asarray(embs, dtype=np.float32))
    assert dense.shape == (B, D) and embs.shape == (B, NUM_EMBS, D)

    nc = _get()
    dsh = dense.reshape(N_CORES, BC, D)
    esh = embs.reshape(N_CORES, BC, NUM_EMBS, D)
    in_maps = [{"dense": dsh[i], "embs": esh[i]} for i in range(N_CORES)]
    res = bass_utils.run_bass_kernel_spmd(nc, in_maps, core_ids=list(range(N_CORES)))
    return np.concatenate([r["out"] for r in res.results], axis=0)

